# revision 13
# baseline (speedup 1.0000x reference)
"""Multi-head attention (B=2, S=2048, D=1024, H=16, HD=64) on 8 trn2 cores.

Sharding: core c = (b, g) with b = c // 4 (batch), g = c % 4 (group of 4
heads).  Each core computes attention for its 4 heads of its batch and a
partial output projection; the host sums the 4 partials per batch and adds
the bias.

v2: bf16 matmul operands (fp32 PSUM accumulate) to cut PE power (the fp32
HIGH-mode run sat at K=4/8 half clock for ~55% of the span) and DMA bytes;
reciprocal_approx_fast instead of the 3.35us-per-call DVE reciprocal;
exact causal clipping (bf16 matmul is full rate at any width, no >=256
floor); DMA spread over 3 HWDGE rings with first-needed tiles first; PE
emission order staggers the RoPE pair-swap matmuls one mm-group behind
their producers so the PSUM->SBUF cast latency hides.

Device-side dataflow (everything in "transposed" layout):
  xT [D, S]  (host-pre-transposed, bf16)
  Q^T = wq_g^T @ xT   [256, S]   (+ RoPE via pair-swap matmul + DVE combine)
  K^T = wk_g^T @ xT   [256, S]   (+ RoPE)
  V = xT-chunk-stationary @ wv -> [S, 64+1(ones)] per head (bf16)
  p^T[k, q] = exp((K^T chunk)^T @ Q^T / 8)  (causal: exact-clipped widths +
                                             additive -1e5 tri on diagonal)
  att^T[d, q], denom[q] = [V | 1]-stationary AV matmul accumulated over k
  att^T *= 1/denom  (reciprocal_approx_fast + gpsimd broadcast + DVE mul)
  out_partial[q, :] = att^T.T @ wo_g  (bf16) -> DRAM
"""

import sys

if "/opt/trn_rl_repo" not in sys.path:
    sys.path.insert(0, "/opt/trn_rl_repo")

import ml_dtypes
import numpy as np

import concourse.bass as bass
import concourse.mybir as mybir
import concourse.tile as tile
from concourse import bacc
from concourse.bass_utils import run_bass_kernel_spmd

FP = mybir.dt.float32
MM = mybir.dt.bfloat16
BF = ml_dtypes.bfloat16

B, S, D, H, HD = 2, 2048, 1024, 16, 64
NCORES = 8
GH = 4  # heads per core
GW = GH * HD  # 256: qkv columns / wo rows per core
ST = 512  # s-tile for projections
NST = S // ST
QT = 512  # q-tile in attention
KC = 128  # k-chunk in attention
DCH = D // 128  # 8 contraction chunks for projections
VW = HD + 2  # 66: per-head V stride (64 + ones col + pad)
NEG = -1.0e5  # additive causal mask (exp(0.125 * NEG) underflows to 0)


def _emit(nc, tc, xT, wq, wk, wv, wo, cosd, sind, trim, out):
    Exp = mybir.ActivationFunctionType.Exp
    PS = bass.MemorySpace.PSUM
    with (
        tc.tile_pool(name="const", bufs=1) as cpool,
        tc.tile_pool(name="wts", bufs=1) as wpool,
        tc.tile_pool(name="qkv", bufs=1) as qpool,
        tc.tile_pool(name="xin", bufs=4) as xpool,
        tc.tile_pool(name="wrk", bufs=2) as wrk,
        tc.tile_pool(name="ptp", bufs=3) as ptp,
        tc.tile_pool(name="obuf", bufs=2) as opool,
        tc.tile_pool(name="psP", bufs=2, space=PS) as psP,
        tc.tile_pool(name="psS", bufs=3, space=PS) as psS,
        tc.tile_pool(name="psAO", bufs=3, space=PS) as psAO,
    ):
        # ---------------- DMA triggers, ordered by first use ----------------
        # scalar ring: wq, wk.  sync ring: xt tiles then out chunks.
        # gpsimd ring: rope tables, wv, tri, wo.
        xTr = xT.ap().rearrange("(c p) s -> p c s", p=128)

        wq_sb = wpool.tile([128, DCH, GW], MM)
        nc.scalar.dma_start(wq_sb[:], wq.ap())
        cos_t = [
            cpool.tile([128, ST], FP, tag=f"cos{i}", name=f"cos_t{i}")
            for i in range(NST)
        ]
        sin_t = [
            cpool.tile([128, ST], FP, tag=f"sin{i}", name=f"sin_t{i}")
            for i in range(NST)
        ]
        nc.scalar.dma_start(cos_t[0][:], cosd.ap()[:, 0:ST])
        nc.scalar.dma_start(sin_t[0][:], sind.ap()[:, 0:ST])

        def load_xt(st):
            ta = xpool.tile([128, DCH // 2, ST], MM, tag="xta")
            nc.sync.dma_start(ta[:], xTr[:, 0:4, st * ST : (st + 1) * ST])
            tb = xpool.tile([128, DCH // 2, ST], MM, tag="xtb")
            nc.sync.dma_start(tb[:], xTr[:, 4:8, st * ST : (st + 1) * ST])
            return ta, tb

        xt0 = load_xt(0)

        wk_sb = wpool.tile([128, DCH, GW], MM)
        nc.gpsimd.dma_start(wk_sb[:], wk.ap())
        wv_sb = wpool.tile([128, DCH, GW], MM)
        nc.gpsimd.dma_start(wv_sb[:], wv.ap())
        tri_sb = cpool.tile([128, 128], FP)
        nc.gpsimd.dma_start(tri_sb[:], trim.ap())
        nc.gpsimd.dma_start(cos_t[1][:], cosd.ap()[:, ST : 2 * ST])
        nc.gpsimd.dma_start(sin_t[1][:], sind.ap()[:, ST : 2 * ST])
        wo_sb = wpool.tile([128, 2, D], MM)

        def late_loads():
            # on the ACT queue: executes after h0(st0)'s exps, keeping the
            # first ~30us of ring bandwidth for start-critical tiles
            for i in range(2, NST):
                nc.scalar.dma_start(cos_t[i][:], cosd.ap()[:, i * ST : (i + 1) * ST])
                nc.scalar.dma_start(sin_t[i][:], sind.ap()[:, i * ST : (i + 1) * ST])
            nc.scalar.dma_start(wo_sb[:], wo.ap())

        # ---------------- persistent activations ----------------
        QTt = qpool.tile([128, 2, S], MM)  # roped Q^T (chunk ch = heads 2ch,2ch+1)
        KTt = qpool.tile([128, 2, S], MM)
        Vt = qpool.tile([128, S // KC, GH * VW], MM)  # [k%128, kc, head-major V]
        attT = qpool.tile([128, 2, S], MM)  # normalized att^T

        ones16 = cpool.tile([128, S // KC], MM)
        nc.vector.memset(ones16[:], 1.0)
        for h in range(GH):
            nc.vector.tensor_copy(Vt[:, :, h * VW + HD], ones16[:])

        # ---------------- QKV projections + RoPE + V transpose ----------------
        # Emission staggers each chunk's rot matmul one mm-group behind its
        # Q/K psum so the DVE cast latency is hidden from the PE queue.
        def proj_mms(wsb, ch, xta, xtb):
            ps = psP.tile([128, ST], FP, tag="proj")
            for dc in range(4):
                nc.tensor.matmul(
                    ps[:],
                    wsb[:, dc, ch * 128 : (ch + 1) * 128],
                    xta[:, dc, :],
                    start=(dc == 0),
                    stop=False,
                )
            for dc in range(4):
                nc.tensor.matmul(
                    ps[:],
                    wsb[:, 4 + dc, ch * 128 : (ch + 1) * 128],
                    xtb[:, dc, :],
                    start=False,
                    stop=(dc == 3),
                )
            return ps

        def rope_cast(ps):
            raw = wrk.tile([128, ST], MM, tag="raw")
            nc.vector.tensor_copy(raw[:], ps[:])
            return raw

        SWAP = []
        for f in range(16):
            SWAP += [2 * f + 1, 2 * f]

        def rope_combine(st, dst, ch, ps, raw):
            ssl = slice(st * ST, (st + 1) * ST)
            shf = wrk.tile([128, ST], MM, tag="shf")
            nc.vector.stream_shuffle(shf[:], raw[:], SWAP)
            t1 = wrk.tile([128, ST], MM, tag="t1")
            nc.vector.tensor_mul(t1[:], ps[:], cos_t[st][:])
            t2 = wrk.tile([128, ST], MM, tag="t2")
            nc.vector.tensor_mul(t2[:], shf[:], sin_t[st][:])
            nc.vector.tensor_add(dst[:, ch, ssl], t1[:], t2[:])

        def v_mms(st, tb, xta, xtb):
            ps = psP.tile([128, GW], FP, tag="proj")
            sl = slice(tb * 128, (tb + 1) * 128)
            for dc in range(4):
                nc.tensor.matmul(
                    ps[:], xta[:, dc, sl], wv_sb[:, dc, :],
                    start=(dc == 0), stop=False,
                )
            for dc in range(4):
                nc.tensor.matmul(
                    ps[:], xtb[:, dc, sl], wv_sb[:, 4 + dc, :],
                    start=False, stop=(dc == 3),
                )
            kc = st * (ST // 128) + tb
            dstap = Vt[:, kc].rearrange("p (h c) -> p h c", c=VW)[:, :, 0:HD]
            nc.vector.tensor_copy(dstap, ps[:].rearrange("p (h c) -> p h c", c=HD))

        def qkv_pieces(st, xt):
            # one closure per PE group; rope combine trails one group behind.
            # Q chunks first: at t=0 wq (scalar ring) lands before wk (gpsimd).
            xta, xtb = xt
            pending = []  # (dst, ch, ps, raw)

            def mk_proj(dst, wsb, ch):
                def go():
                    ps = proj_mms(wsb, ch, xta, xtb)
                    raw = rope_cast(ps)
                    if pending:
                        rope_combine(st, *pending.pop())
                    pending.append((dst, ch, ps, raw))

                return go

            def mk_v(tb):
                def go():
                    v_mms(st, tb, xta, xtb)
                    if pending:
                        rope_combine(st, *pending.pop())

                return go

            return [
                mk_proj(QTt, wq_sb, 0),
                mk_proj(QTt, wq_sb, 1),
                mk_proj(KTt, wk_sb, 0),
                mk_proj(KTt, wk_sb, 1),
                mk_v(0),
                mk_v(1),
                mk_v(2),
                mk_v(3),
            ]

        def qkv_st(st, xt):
            for piece in qkv_pieces(st, xt):
                piece()

        # ---------------- attention (one head, one q-tile) ----------------
        def attn_head(st, h, fillers=(), cadence=0):
            qs = st * QT
            nkc = (qs + QT) // KC
            ch, r0 = h // 2, (h % 2) * HD
            aps = psAO.tile([HD + 1, QT], FP, tag="ao")
            prev = None  # (kc, off, pt)

            def av(kc, off, pt):
                nc.tensor.matmul(
                    aps[:, off:QT],
                    Vt[:, kc, h * VW : h * VW + HD + 1],
                    pt[:, 0 : QT - off],
                    start=(kc == 0),
                    stop=(kc == nkc - 1),
                )

            for kc in range(nkc):
                ks = kc * KC
                off = max(0, ks - qs)
                w = QT - off
                sc = psS.tile([128, QT], FP, tag="sc")
                nc.tensor.matmul(
                    sc[:, 0:w],
                    KTt[r0 : r0 + HD, ch, ks : ks + KC],
                    QTt[r0 : r0 + HD, ch, qs + off : qs + QT],
                    start=True,
                    stop=True,
                )
                if ks >= qs:  # diagonal block: mask k > q additively
                    nc.vector.tensor_add(sc[:, 0:KC], sc[:, 0:KC], tri_sb[:])
                pt = ptp.tile([128, QT], MM, tag="pt")
                nc.scalar.activation(pt[:, 0:w], sc[:, 0:w], Exp, scale=0.125)
                if prev is not None:
                    av(*prev)
                if fillers and cadence and kc % cadence == cadence - 1:
                    fillers.pop(0)()
                prev = (kc, off, pt)
            av(*prev)

            dcp = wrk.tile([1, QT], FP, tag="dcp")
            nc.vector.tensor_copy(dcp[:], aps[HD : HD + 1, :])
            rec = wrk.tile([1, QT], FP, tag="rec")
            nc.vector.reciprocal_approx_fast(rec[:], dcp[:])
            bcs = wrk.tile([HD, QT], FP, tag="bcs")
            nc.gpsimd.partition_broadcast(bcs[:], rec[:])
            nc.vector.tensor_mul(
                attT[r0 : r0 + HD, ch, qs : qs + QT], aps[0:HD, :], bcs[:]
            )

        # ---------------- output projection (one 128-row block) ----------------
        def outproj_qb(qb):
            ob = opool.tile([128, D], MM, tag="ob")
            qsl = slice(qb * 128, (qb + 1) * 128)
            for nt in range(2):
                ops = psAO.tile([128, 512], FP, tag="ao")
                nsl = slice(nt * 512, (nt + 1) * 512)
                nc.tensor.matmul(
                    ops[:], attT[:, 0, qsl], wo_sb[:, 0, nsl],
                    start=True, stop=False,
                )
                nc.tensor.matmul(
                    ops[:], attT[:, 1, qsl], wo_sb[:, 1, nsl],
                    start=False, stop=True,
                )
                nc.vector.tensor_copy(ob[:, nsl], ops[:])
            nc.sync.dma_start(out[qsl, :], ob[:])

        def outproj_pieces(qt):
            def mk(qb):
                return lambda: outproj_qb(qb)

            return [mk(qb) for qb in range(qt * (QT // 128), (qt + 1) * (QT // 128))]

        def outproj_qt(qt):
            for piece in outproj_pieces(qt):
                piece()

        # fused pipeline: outproj of st-1 leads stile st (its attT inputs
        # finished during st-1, so the PE never waits on it); attention for
        # qt=st runs right after qkv of st with scores one chunk ahead of AV.
        qkv_st(0, xt0)
        for st in range(NST):
            xt_next = load_xt(st + 1) if st + 1 < NST else None
            fillers = []
            if st + 1 < NST:
                fillers += qkv_pieces(st + 1, xt_next)
            if st > 0:
                fillers += outproj_pieces(st - 1)
            nchunks = GH * ((st * QT + QT) // KC)
            cadence = max(1, nchunks // len(fillers) + 2) if fillers else 0
            for h in range(GH):
                attn_head(st, h, fillers, cadence)
                if st == 0 and h == 0:
                    late_loads()
            for piece in fillers:
                piece()
        outproj_qt(NST - 1)


_prog = None


def _build():
    global _prog
    if _prog is not None:
        return _prog
    nc = bacc.Bacc("TRN2", target_bir_lowering=False, debug=False)
    xT = nc.declare_dram_parameter("xT", [D, S], MM, isOutput=False)
    wq = nc.declare_dram_parameter("wq", [128, DCH, GW], MM, isOutput=False)
    wk = nc.declare_dram_parameter("wk", [128, DCH, GW], MM, isOutput=False)
    wv = nc.declare_dram_parameter("wv", [128, DCH, GW], MM, isOutput=False)
    wo = nc.declare_dram_parameter("wo", [128, 2, D], MM, isOutput=False)
    cosd = nc.declare_dram_parameter("cosd", [128, S], FP, isOutput=False)
    sind = nc.declare_dram_parameter("sind", [128, S], FP, isOutput=False)
    trim = nc.declare_dram_parameter("trim", [128, 128], FP, isOutput=False)
    out = nc.declare_dram_parameter("out", [S, D], MM, isOutput=True)
    with tile.TileContext(nc) as tc:
        _emit(nc, tc, xT, wq, wk, wv, wo, cosd, sind, trim, out)
    nc.compile()
    _prog = nc
    return nc


def _tables():
    inv = 1.0 / (10000.0 ** (np.arange(0, HD, 2)[: HD // 2].astype(np.float32) / HD))
    ang = np.outer(np.arange(S, dtype=np.float32), inv).astype(np.float32)  # [S, 32]
    cos64 = np.repeat(np.cos(ang).T, 2, axis=0).astype(np.float32)  # [64, S]
    sin64 = np.repeat(np.sin(ang).T, 2, axis=0).astype(np.float32)
    sin64[0::2, :] *= -1.0  # out[2f] = x[2f]c - x[2f+1]s (swap partner sign)
    cos128 = np.tile(cos64, (2, 1))
    sin128 = np.tile(sin64, (2, 1))
    kk, qq = np.meshgrid(np.arange(128), np.arange(128), indexing="ij")
    tri = np.where(kk <= qq, 0.0, NEG).astype(np.float32)  # additive causal mask
    return cos128, sin128, tri


def _pack_w(w):  # [D, GW] -> [128, DCH, GW], row dc*128+p -> [p, dc]
    return np.ascontiguousarray(
        np.asarray(w, np.float32).reshape(DCH, 128, GW).transpose(1, 0, 2)
    ).astype(BF)


def _pack_wo(w):  # [GW, D] -> [128, 2, D]
    return np.ascontiguousarray(
        np.asarray(w, np.float32).reshape(2, 128, D).transpose(1, 0, 2)
    ).astype(BF)


def make_in_maps(x, wq, wk, wv, wo_w):
    cos128, sin128, tri = _tables()
    in_maps = []
    for c in range(NCORES):
        b, g = divmod(c, GH)
        cs = slice(g * GW, (g + 1) * GW)
        in_maps.append(
            {
                "xT": np.ascontiguousarray(np.asarray(x[b], np.float32).T).astype(BF),
                "wq": _pack_w(wq[:, cs]),
                "wk": _pack_w(wk[:, cs]),
                "wv": _pack_w(wv[:, cs]),
                "wo": _pack_wo(wo_w[cs, :]),
                "cosd": cos128,
                "sind": sin128,
                "trim": tri,
            }
        )
    return in_maps


def kernel(x, wq, wk, wv, wo_w, wo_b):
    nc = _build()
    in_maps = make_in_maps(x, wq, wk, wv, wo_w)
    res = run_bass_kernel_spmd(nc, in_maps, list(range(NCORES))).results
    out = np.zeros((B, S, D), np.float32)
    for c in range(NCORES):
        out[c // GH] += np.asarray(res[c]["out"], dtype=np.float32)
    out += np.asarray(wo_b, np.float32)[None, None, :]
    return out
